# revision 2
# baseline (speedup 1.0000x reference)
"""BiDirectional LSTM (B=32, T=512, D=H=512, hard_sigmoid gates, output=fwd+bwd sum)
on 8 Trainium2 NeuronCores — v2.

Sharding: core c in 0..7 -> direction d = c//4 (0=fwd, 1=bwd), batch shard s = c%4
(8 samples each). The bwd cores receive time-reversed x; scan outputs stack in
iteration order (Theano go_backwards semantics), so fwd+bwd partials add at equal
step indices.

v2 changes vs v1:
  - xz (input-gate projections) kept entirely in SBUF as bf16 — no DRAM round trip,
    no per-step DMA, no DMA semaphores in the recurrence.
  - Sigmoid gates pre-scaled on host: W',U' *= 0.2, b' = 0.2b + 0.5, so
    hard_sigmoid(z) = clip(z', 0, 1) = ONE dual-op tensor_scalar (min,max) on DVE.
  - Gate column order [i, c~, f, o]: i/c~ pre-activations finish first (their
    nonlinearities + c-chain overlap f/o matmuls), o last so the step tail is only
    zadd_o -> clip_o -> y mul.
  - y stored bf16 in SBUF and doubles as the h history read by next step's matmuls
    (no separate h copy). One bulk DMA of y at the end.
  - ACT engine does only Tanh (+ phase-1 Identity-bias copies) — single act table.
"""

import numpy as np
import ml_dtypes

B, T, D, H = 32, 512, 512, 512
NCORES = 8
BC = B // 4          # 8 samples per core
KT = D // 128        # 4 k-tiles
MT = (4 * H) // 128  # 16 m-tiles (4 gates x 4 chunks), order [i, c, f, o]


def build(nc, Tn=T):
    import concourse.mybir as mybir
    from concourse.tile import TileContext, add_dep_helper

    f32 = mybir.dt.float32
    bf16 = mybir.dt.bfloat16
    AF = mybir.ActivationFunctionType
    OP = mybir.AluOpType
    NT = Tn * BC        # GEMM moving size (t,b) flattened
    NCK = min(256, NT)  # phase-1 n-chunk width
    NCH = NT // NCK     # number of chunks
    TCH = NCK // BC     # timesteps per chunk

    xT = nc.declare_dram_parameter("xT", [KT, 128, NT], bf16, isOutput=False)
    w = nc.declare_dram_parameter("w", [KT, 128, 4 * H], bf16, isOutput=False)
    u = nc.declare_dram_parameter("u", [KT, 128, 4 * H], bf16, isOutput=False)
    bias = nc.declare_dram_parameter("bias", [128, MT], f32, isOutput=False)
    y = nc.declare_dram_parameter("y", [128, Tn, KT, BC], bf16, isOutput=True)

    with TileContext(nc) as tc:
        with (
            tc.tile_pool(name="const", bufs=1) as cpool,
            tc.tile_pool(name="state", bufs=1) as spool,
            tc.tile_pool(name="xcpool", bufs=2) as xcpool,
            tc.tile_pool(name="rpsum", bufs=2, space="PSUM") as rpsum,
            tc.tile_pool(name="ztmp", bufs=2) as zpool,
        ):
            u_sb = [cpool.tile([128, 4 * H], bf16, name=f"u{k}", tag=f"u{k}") for k in range(KT)]
            w_sb = [cpool.tile([128, 4 * H], bf16, name=f"w{k}", tag=f"w{k}") for k in range(KT)]
            bias_sb = cpool.tile([128, MT], f32, name="bias", tag="bias")
            for k in range(KT):
                nc.sync.dma_start(out=w_sb[k], in_=w[k])
            for k in range(KT):
                nc.sync.dma_start(out=u_sb[k], in_=u[k])
            nc.sync.dma_start(out=bias_sb, in_=bias[:])

            # Chunked SBUF state: xz and y in NCH chunks of TCH steps each, so
            # phase-1 pieces and y write-back DMAs interleave with the
            # recurrence at chunk granularity (tile-level deps stay precise).
            xz_ch = [
                spool.tile([128, MT, NCK], bf16, name=f"xz{c}", tag=f"xz{c}")
                for c in range(NCH)
            ]
            y_ch = [
                spool.tile([128, TCH, KT, BC], bf16, name=f"y{c}", tag=f"y{c}")
                for c in range(NCH)
            ]
            c_st = spool.tile([128, KT, BC], f32, name="c_st", tag="c_st")

            def xz_t(t, lo, hi):
                return xz_ch[t // TCH][:, lo:hi, (t % TCH) * BC : (t % TCH + 1) * BC]

            def y_at(t):
                return y_ch[t // TCH][:, t % TCH]

            # ---- phase-1 pieces: one m-tile of one chunk (4 MMs + ACT copy).
            # Chunk 0 runs upfront; the rest interleave into the recurrence
            # (PE work fits in the per-step tail idle, ACT copy ~570ns).
            xck_tiles = {}

            def fetch_xck(cj):
                xck = xcpool.tile([128, KT, NCK], bf16, name="xck", tag="xck")
                for k in range(KT):
                    nc.sync.dma_start(
                        out=xck[:, k], in_=xT[k][:, cj * NCK : (cj + 1) * NCK]
                    )
                xck_tiles[cj] = xck

            def emit_piece(j):
                cj, m = divmod(j, MT)
                if m == 0 and cj + 1 < NCH:
                    fetch_xck(cj + 1)  # prefetch next chunk's x one chunk ahead
                ps = rpsum.tile([128, NCK], f32, name="pps", tag=f"ps{j % 4}")
                for k in range(KT):
                    nc.tensor.matmul(
                        ps,
                        lhsT=w_sb[k][:, m * 128 : (m + 1) * 128],
                        rhs=xck_tiles[cj][:, k],
                        start=(k == 0),
                        stop=(k == KT - 1),
                    )
                nc.scalar.activation(
                    xz_ch[cj][:, m, :], ps,
                    AF.Identity, bias=bias_sb[:, m : m + 1], scale=1.0,
                )

            fetch_xck(0)
            for j in range(MT):  # chunk 0 upfront (also sets has_written on
                emit_piece(j)    # all 8 PSUM bank slots via start=True MMs)
            npieces = NCH * MT

            # ---- step 0: h = 0, z = xz only ----
            sig_i = zpool.tile([128, KT, BC], f32, name="sig_i", tag="sig_i")
            g_t = zpool.tile([128, KT, BC], f32, name="g_t", tag="g_t")
            sig_o = zpool.tile([128, KT, BC], f32, name="sig_o", tag="sig_o")
            th = zpool.tile([128, KT, BC], f32, name="th", tag="th")
            nc.vector.tensor_scalar(sig_i, xz_t(0, 4, 8), 1.0, 0.0, OP.min, OP.max)
            nc.scalar.activation(g_t, xz_t(0, 0, 4), AF.Tanh)
            nc.vector.tensor_mul(c_st, sig_i, g_t)
            th_prev = nc.scalar.activation(th, c_st, AF.Tanh)
            nc.vector.tensor_scalar(sig_o, xz_t(0, 12, 16), 1.0, 0.0, OP.min, OP.max)
            nc.vector.tensor_mul(y_at(0), sig_o, th)

            piece_j = MT
            for t in range(1, Tn):
                # PSUM preloads on ACT, ordered after the previous tanh(c)
                psg = [
                    rpsum.tile([128, KT, BC], f32, name=f"ps{g}", tag=f"ps{g}")
                    for g in range(4)
                ]
                for g in range(4):
                    pre = nc.scalar.activation(
                        psg[g], xz_t(t, 4 * g, 4 * g + 4), AF.Copy
                    )
                    add_dep_helper(pre.ins, th_prev.ins, sync=False,
                                   reason="preload after prev tanh(c)")
                # interleaved phase-1 piece (every other step)
                if t % 2 == 1 and piece_j < npieces:
                    emit_piece(piece_j)
                    piece_j += 1
                for g in range(4):
                    for mi in range(KT):
                        m = g * 4 + mi
                        for k in range(KT):
                            nc.tensor.matmul(
                                psg[g][:, mi, :],
                                lhsT=u_sb[k][:, m * 128 : (m + 1) * 128],
                                rhs=y_at(t - 1)[:, k, :],
                                start=False,
                                stop=(k == KT - 1),
                            )
                sig_i = zpool.tile([128, KT, BC], f32, name="sig_i", tag="sig_i")
                g_t = zpool.tile([128, KT, BC], f32, name="g_t", tag="g_t")
                sig_f = zpool.tile([128, KT, BC], f32, name="sig_f", tag="sig_f")
                sig_o = zpool.tile([128, KT, BC], f32, name="sig_o", tag="sig_o")
                t1 = zpool.tile([128, KT, BC], f32, name="t1", tag="t1")
                t2 = zpool.tile([128, KT, BC], f32, name="t2", tag="t2")
                th = zpool.tile([128, KT, BC], f32, name="th", tag="th")
                # c~ (gate group 1): tanh(g) ACT round trip overlaps i/f/o MMs
                nc.scalar.activation(g_t, psg[0], AF.Tanh)
                # i (group 2)
                nc.vector.tensor_scalar(sig_i, psg[1], 1.0, 0.0, OP.min, OP.max)
                i_t2 = nc.vector.tensor_mul(t2, sig_i, g_t)
                # f (group 3) -> c update -> tanh(c) during o MMs
                i_cf = nc.vector.tensor_scalar(sig_f, psg[2], 1.0, 0.0, OP.min, OP.max)
                add_dep_helper(i_cf.ins, i_t2.ins, sync=False,
                               reason="keep t2 ahead of clip_f on DVE")
                nc.vector.tensor_mul(t1, sig_f, c_st)
                i_ca = nc.vector.tensor_add(c_st, t1, t2)
                th_prev = nc.scalar.activation(th, c_st, AF.Tanh)
                # o tail
                i_co = nc.vector.tensor_scalar(sig_o, psg[3], 1.0, 0.0, OP.min, OP.max)
                add_dep_helper(i_co.ins, i_ca.ins, sync=False,
                               reason="keep c_add ahead of clip_o on DVE")
                nc.vector.tensor_mul(y_at(t), sig_o, th)
                if (t + 1) % TCH == 0:
                    c = t // TCH
                    nc.sync.dma_start(
                        out=y[:, c * TCH : (c + 1) * TCH], in_=y_ch[c]
                    )
    return nc


def _prep_core_inputs(x, weights, core, Tn=T):
    """weights: dict with all 24 weight arrays (np float32)."""
    d = core // 4
    s = core % 4
    pre = "" if d == 0 else "b"
    # gate order [c, i, f, o]; sigmoid gates pre-scaled by 0.2 (+0.5 into bias)
    gates = ["c", "i", "f", "o"]
    scale = {"i": 0.2, "c": 1.0, "f": 0.2, "o": 0.2}
    shift = {"i": 0.5, "c": 0.0, "f": 0.5, "o": 0.5}
    Wc = np.concatenate([weights[f"W{pre}_{g}"] * scale[g] for g in gates], axis=1)
    Uc = np.concatenate([weights[f"U{pre}_{g}"] * scale[g] for g in gates], axis=1)
    bc = np.concatenate(
        [weights[f"b{pre}_{g}"] * scale[g] + shift[g] for g in gates], axis=0
    )
    xc = x[s * BC : (s + 1) * BC, :Tn]
    if d == 1:
        xc = xc[:, ::-1]
    # [b, t, d] -> [d, t, b] -> [KT, 128, Tn*BC]
    xTc = np.ascontiguousarray(xc.transpose(2, 1, 0)).reshape(KT, 128, Tn * BC)
    return {
        "xT": xTc.astype(ml_dtypes.bfloat16),
        "w": Wc.reshape(KT, 128, 4 * H).astype(ml_dtypes.bfloat16),
        "u": Uc.reshape(KT, 128, 4 * H).astype(ml_dtypes.bfloat16),
        "bias": np.ascontiguousarray(bc.reshape(MT, 128).T).astype(np.float32),
    }


def _gather(results, Tn=T):
    out = np.empty((B, Tn, H), np.float32)
    for s in range(4):
        acc = None
        for d in range(2):
            yc = np.asarray(results[d * 4 + s]["y"], dtype=np.float32)  # [128, Tn, KT, BC]
            part = yc.transpose(3, 1, 2, 0).reshape(BC, Tn, H)
            acc = part if acc is None else acc + part
        out[s * BC : (s + 1) * BC] = acc
    return out


def run(inputs, Tn=T, trace=False):
    import concourse.bacc as bacc
    from concourse.bass_utils import run_bass_kernel_spmd

    x = np.asarray(inputs["x"], np.float32)
    weights = {k: np.asarray(v, np.float32) for k, v in inputs.items() if k != "x"}
    nc = bacc.Bacc("TRN2", target_bir_lowering=False)
    build(nc, Tn)
    nc.compile()
    in_maps = [_prep_core_inputs(x, weights, c, Tn) for c in range(NCORES)]
    res = run_bass_kernel_spmd(nc, in_maps, list(range(NCORES)), trace=trace)
    return _gather(res.results, Tn), res


def kernel(**inputs):
    out, _ = run(inputs)
    return out
